# revision 12
# baseline (speedup 1.0000x reference)
"""AttentionPool kernel for Trainium2, 8 NeuronCores (SPMD data-parallel).

Reference computation (per graph g with atoms A_g, uniform |A_g| = 32):
    h = X @ W.T                              [131072, 512]
    s = leakyrelu(sum(att * h, -1), 0.2)     [131072]
    w = segment_softmax(s)                   per graph
    out[g] = sum_{a in A_g} w[a] * h[a]      [4096, 512]

Algebraic refactor (pool-first; avoids the 69-GFLOP h matmul AND any
transpose of X):
    v  = W.T @ att  (host input prep, tiny)
    s  = lrelu(X @ v)        fused per-tile dot product (DVE/GpSimd stt)
    e  = exp(s)              no max-subtraction needed (|s| <~ 8)
    P[g] = sum_{a in A_g} e[a] X[a]   PE matmul per 128-atom tile with a
                             [128,32] masked-exp stationary em32 built by
                             ACT: exp(maskbias + s); 8-tile batches write
                             a 32-aligned PSUM partition window
    d[g] = per-tile matmul em32.T @ ones (ap_size=1, ~free on PE)
    out = (P / d) @ W.T      per-core projection (PE transposes + matmul)

All heavy data in bf16 (X converted host-side -> 17 MB DMA per core,
matmuls at 1 cyc/row); s/den/PSUM accumulate in fp32. Rel err ~1e-3
vs the 2e-2 gate.

Sharding: 8 cores x 16384 atoms (= 512 graphs, graph-aligned). W/att
replicated. Output slices concatenated on host. Non-uniform segment sizes
fall back to an exact numpy path (never triggered by the fixed harness
inputs, which are uniform 32 atoms/graph).
"""

import numpy as np

N_ATOMS = 131072
FEAT = 512
N_GRAPHS = 4096
NEG_SLOPE = 0.2
N_CORES = 8

P = 128                      # partitions / atoms per tile
NA_CORE = N_ATOMS // N_CORES         # 16384 atoms per core
NT = NA_CORE // P                    # 128 tiles per core
NG_CORE = N_GRAPHS // N_CORES        # 512 graphs per core
GPT = P // 32                        # 4 graphs per tile (uniform 32 atoms/graph)
TPG = P // GPT                       # 32 tiles per 128-graph group
NGRP = NT // TPG                     # 4 groups of 128 graphs per core
FCH = FEAT // P                      # 4 feature chunks
DMA_GRP = 8                          # X tiles per input DMA (1 MiB in bf16)
W32 = 8 * GPT                        # stationary width = graphs per 8-tile batch
GP_KS = (2, 6)                       # tiles per 8-batch whose product runs on GpSimd

_CACHED = {}


def _build_program():
    import concourse.bacc as bacc
    import concourse.mybir as mybir
    import concourse.tile as tile
    from concourse.masks import make_identity
    from contextlib import ExitStack

    F32 = mybir.dt.float32
    BF16 = mybir.dt.bfloat16
    FP16 = mybir.dt.float16
    MULT = mybir.AluOpType.mult
    ADD = mybir.AluOpType.add
    MAX = mybir.AluOpType.max
    EXP = mybir.ActivationFunctionType.Exp

    nc = bacc.Bacc("TRN2", target_bir_lowering=False, debug=False,
                   num_devices=N_CORES)

    x = nc.dram_tensor("x", [NA_CORE, FEAT], BF16, kind="ExternalInput").ap()
    wt = nc.dram_tensor("wt", [FEAT, FEAT], BF16, kind="ExternalInput").ap()
    vrep = nc.dram_tensor("vrep", [P, FEAT], BF16, kind="ExternalInput").ap()
    mb32 = nc.dram_tensor("mb32", [P, 2 * W32 - GPT], F32,
                          kind="ExternalInput").ap()
    out = nc.dram_tensor("out", [NG_CORE, FEAT], F32, kind="ExternalOutput").ap()

    x_r8 = x.rearrange("(n o p) f -> n p o f", o=DMA_GRP, p=P)
    x_r4 = x.rearrange("(n o p) f -> n p o f", o=4, p=P)

    with tile.TileContext(nc) as tc, ExitStack() as ctx:
        singles = ctx.enter_context(tc.tile_pool(name="singles", bufs=1))
        xpool = ctx.enter_context(tc.tile_pool(name="xpool", bufs=6))
        x4pool = ctx.enter_context(tc.tile_pool(name="x4pool", bufs=2))
        spool = ctx.enter_context(tc.tile_pool(name="spool", bufs=6))
        prodv = ctx.enter_context(tc.tile_pool(name="prodv", bufs=4))
        prodg = ctx.enter_context(tc.tile_pool(name="prodg", bufs=3))
        junkp = ctx.enter_context(tc.tile_pool(name="junkp", bufs=2))
        empool = ctx.enter_context(tc.tile_pool(name="empool", bufs=8))
        smallp = ctx.enter_context(tc.tile_pool(name="smallp", bufs=4))
        pooledp = ctx.enter_context(tc.tile_pool(name="pooledp", bufs=2))
        ptp = ctx.enter_context(tc.tile_pool(name="ptp", bufs=4))
        outp = ctx.enter_context(tc.tile_pool(name="outp", bufs=2))
        ps_pool = ctx.enter_context(tc.tile_pool(name="ps_pool", bufs=2, space="PSUM"))
        ps_den = ctx.enter_context(tc.tile_pool(name="ps_den", bufs=2, space="PSUM"))
        ps_misc = ctx.enter_context(tc.tile_pool(name="ps_misc", bufs=2, space="PSUM"))
        ps_out = ctx.enter_context(tc.tile_pool(name="ps_out", bufs=2, space="PSUM"))

        # ---- first X chunk + constants, then the streaming 8-tile chunks ----
        xc0 = x4pool.tile([P, 4, FEAT], BF16, tag="x4i", name="xc0")
        nc.sync.dma_start(out=xc0, in_=x_r4[0])
        v_rep = singles.tile([P, FEAT], BF16)
        nc.sync.dma_start(out=v_rep, in_=vrep)
        xc1 = x4pool.tile([P, 4, FEAT], BF16, tag="x4i", name="xc1")
        nc.sync.dma_start(out=xc1, in_=x_r4[1])
        mb32_sb = singles.tile([P, 2 * W32 - GPT], F32)
        nc.sync.dma_start(out=mb32_sb, in_=mb32)
        wt_sb = singles.tile([P, FCH, FEAT], BF16)
        nc.sync.dma_start(out=wt_sb, in_=wt.rearrange("(c p) f -> p c f", p=P))
        ident = singles.tile([P, P], BF16)
        make_identity(nc, ident)
        ones_col = singles.tile([P, 1], BF16)
        nc.vector.memset(ones_col, 1.0)

        # warm up the PE (HAM clock gate) while the score pipeline fills
        warm_ps = ps_out.tile([P, FEAT], F32, tag="ops", name="warm_ps")
        for wi in range(6):
            nc.tensor.matmul(warm_ps, lhsT=ident, rhs=v_rep,
                             start=(wi == 0), stop=(wi == 5))

        state = {"chunk": None}

        def xref(t):
            """SBUF AP for tile t; issues the owning chunk DMA at boundaries."""
            if t < 4:
                return xc0[:, t, :]
            if t < 8:
                return xc1[:, t - 4, :]
            n, o = divmod(t, DMA_GRP)
            if o == 0:
                state["chunk"] = xpool.tile([P, DMA_GRP, FEAT], BF16, tag="x8",
                                            name=f"x8_{n}")
                nc.sync.dma_start(out=state["chunk"], in_=x_r8[n])
            return state["chunk"][:, o, :]

        def emit_batch(g, bu, pool_ps, den_ps):
            s_b = spool.tile([P, 8], F32, tag="s_b")
            xts = [xref(g * TPG + bu * 8 + k) for k in range(8)]
            prods = {}
            for k in GP_KS:
                pg = prodg.tile([P, FEAT], FP16, tag="pg")
                nc.gpsimd.tensor_tensor(out=pg, in0=xts[k], in1=v_rep, op=MULT)
                prods[k] = pg
            dve_ks = [k for k in range(8) if k not in GP_KS]
            for k in dve_ks:
                pv = prodv.tile([P, FEAT], FP16, tag="pv")
                nc.vector.tensor_tensor(out=pv, in0=xts[k], in1=v_rep, op=MULT)
                prods[k] = pv
            for k in dve_ks + list(GP_KS):
                junk = junkp.tile([P, FEAT], FP16, tag="junk")
                nc.vector.tensor_scalar(
                    out=junk, in0=prods[k], scalar1=1.0, scalar2=0.0,
                    op0=MULT, op1=ADD, accum_out=s_b[:, k:k + 1])
            s_lr = spool.tile([P, 8], F32, tag="s_lr")
            nc.vector.scalar_tensor_tensor(
                out=s_lr, in0=s_b, scalar=NEG_SLOPE, in1=s_b,
                op0=MULT, op1=MAX)
            win = pool_ps[bu * W32:(bu + 1) * W32, :]
            dwin = den_ps[bu * W32:(bu + 1) * W32, :]
            for k in range(8):
                # em32[p, c] = exp(s_lr[p,k]) iff c == 4k + p//32 else 0
                em = empool.tile([P, W32], BF16, tag="em")
                nc.scalar.activation(out=em,
                                     in_=mb32_sb[:, W32 - GPT - GPT * k:
                                                 2 * W32 - GPT - GPT * k],
                                     func=EXP,
                                     bias=s_lr[:, k:k + 1], scale=1.0)
                nc.tensor.matmul(win, lhsT=em, rhs=xts[k],
                                 start=(k == 0), stop=(k == 7),
                                 tile_position=(0, bu * W32))
                nc.tensor.matmul(dwin, lhsT=em, rhs=ones_col,
                                 start=(k == 0), stop=(k == 7),
                                 tile_position=(0, bu * W32))

        def emit_tail(g, pool_ps, den_ps):
            # ---- normalize + projection: out[g] = (pool/den) @ W.T ----
            denr = smallp.tile([P, 1], F32, tag="denr")
            nc.vector.reciprocal(denr, den_ps)
            pooled = pooledp.tile([P, FEAT], BF16, tag="pooled")
            out_ps = ps_out.tile([P, FEAT], F32, tag="ops")
            for c in range(FCH):
                nc.scalar.mul(pooled[:, c * P:(c + 1) * P],
                              pool_ps[:, c * P:(c + 1) * P], denr)
                tr_ps = ps_misc.tile([P, P], BF16, tag="tr", name="tr_ps")
                nc.tensor.transpose(tr_ps,
                                    pooled[:, c * P:(c + 1) * P],
                                    ident)
                pt = ptp.tile([P, P], BF16, tag="pt")
                nc.scalar.copy(out=pt, in_=tr_ps)
                nc.tensor.matmul(out_ps, lhsT=pt,
                                 rhs=wt_sb[:, c, :],
                                 start=(c == 0), stop=(c == FCH - 1))
            out_sb = outp.tile([P, FEAT], F32, tag="out_sb")
            nc.scalar.copy(out=out_sb, in_=out_ps)
            nc.sync.dma_start(out=out[g * P:(g + 1) * P, :], in_=out_sb)

        # ---- main loop: 4 groups x 4 batches x 8 tiles, tails pipelined ----
        prev = None
        for g in range(NGRP):
            pool_ps = ps_pool.tile([P, FEAT], F32)
            den_ps = ps_den.tile([P, 1], F32)
            for bu in range(TPG // 8):
                emit_batch(g, bu, pool_ps, den_ps)
                if bu == 0 and prev is not None:
                    emit_tail(*prev)
            prev = (g, pool_ps, den_ps)
        emit_tail(*prev)
    nc.compile()
    return nc


def _host_inputs(atomwise_output, W, att_weight):
    """Per-core input maps (host prep: bf16 conversion + tiny mask tables)."""
    import ml_dtypes
    BF = ml_dtypes.bfloat16
    X = np.asarray(atomwise_output, dtype=np.float32)
    Xb = X.astype(BF)
    Wc = np.ascontiguousarray(np.asarray(W, dtype=np.float32))
    Wt = np.ascontiguousarray(Wc.T)
    att = np.asarray(att_weight, dtype=np.float32)
    v = Wt @ att                                               # v = W.T @ att
    Wtb = Wt.astype(BF)
    vrep = np.ascontiguousarray(np.broadcast_to(v, (P, FEAT))).astype(BF)
    # master mask-bias: mb32[p, c] = 0 iff c == (W32 - GPT) + p//32; the
    # per-tile variant k is the window mb32[:, (W32-GPT)-GPT*k : (2*W32-GPT)-GPT*k]
    pp = np.arange(P)[:, None]
    cc = np.arange(2 * W32 - GPT)[None, :]
    mb = np.where(cc == (W32 - GPT) + pp // 32, 0.0, -1e9).astype(np.float32)
    mb = np.ascontiguousarray(mb)
    in_maps = []
    for c in range(N_CORES):
        xc = Xb[c * NA_CORE:(c + 1) * NA_CORE]
        in_maps.append({"x": xc, "wt": Wtb, "vrep": vrep, "mb32": mb})
    return in_maps


def _kernel_numpy_fallback(atomwise_output, n_atoms_i, W, att_weight):
    """Exact reference semantics in numpy (used only for non-uniform segments)."""
    X = np.asarray(atomwise_output, dtype=np.float32)
    n_at = np.asarray(n_atoms_i).astype(np.int64)
    W = np.asarray(W, dtype=np.float32)
    att = np.asarray(att_weight, dtype=np.float32)
    h = X @ W.T
    s = (att * h).sum(-1)
    s = np.where(s >= 0, s, NEG_SLOPE * s)
    seg = np.repeat(np.arange(len(n_at)), n_at)[:len(s)]
    ngr = len(n_at)
    smax = np.full(ngr, -np.inf, dtype=np.float32)
    np.maximum.at(smax, seg, s)
    e = np.exp(s - smax[seg])
    den = np.zeros(ngr, dtype=np.float32)
    np.add.at(den, seg, e)
    wgt = e / den[seg]
    outp = np.zeros((ngr, h.shape[1]), dtype=np.float32)
    np.add.at(outp, seg, wgt[:, None] * h)
    return outp


def _run_on_device(atomwise_output, W, att_weight):
    from concourse.bass_utils import run_bass_kernel_spmd

    if "nc" not in _CACHED:
        _CACHED["nc"] = _build_program()
    nc = _CACHED["nc"]
    in_maps = _host_inputs(atomwise_output, W, att_weight)
    res = run_bass_kernel_spmd(nc, in_maps, list(range(N_CORES)))
    return np.concatenate([res.results[c]["out"] for c in range(N_CORES)], axis=0)


def _run_in_subprocess(atomwise_output, n_atoms_i, W, att_weight):
    """Last-resort retry in a fresh process: a transient
    NRT_EXEC_UNIT_UNRECOVERABLE wedges the current NRT client session, but a
    new process (fresh axon boot) recovers. Arrays go via a temp dir."""
    import os, subprocess, sys, tempfile
    kdir = os.path.dirname(os.path.abspath(__file__))
    with tempfile.TemporaryDirectory() as td:
        np.save(os.path.join(td, "x.npy"), np.asarray(atomwise_output))
        np.save(os.path.join(td, "n.npy"), np.asarray(n_atoms_i))
        np.save(os.path.join(td, "w.npy"), np.asarray(W))
        np.save(os.path.join(td, "a.npy"), np.asarray(att_weight))
        driver = (
            "import sys, os, numpy as np\n"
            f"sys.path.insert(0, {kdir!r})\n"
            "import kernel\n"
            f"td = {td!r}\n"
            "out = kernel.kernel(np.load(td+'/x.npy'), np.load(td+'/n.npy'),\n"
            "                    np.load(td+'/w.npy'), np.load(td+'/a.npy'))\n"
            "np.save(td+'/out.npy', out)\n"
        )
        env = dict(os.environ, KERNEL_NO_SUBPROC="1")
        subprocess.run([sys.executable, "-c", driver], env=env, check=True,
                       timeout=1800)
        return np.load(os.path.join(td, "out.npy"))


def kernel(atomwise_output, n_atoms_i, W, att_weight):
    import os
    n_at = np.asarray(n_atoms_i)
    uniform = (
        atomwise_output.shape == (N_ATOMS, FEAT)
        and n_at.shape == (N_GRAPHS,)
        and np.all(n_at == N_ATOMS // N_GRAPHS)
    )
    if not uniform:
        return _kernel_numpy_fallback(atomwise_output, n_atoms_i, W, att_weight)

    try:
        out = _run_on_device(atomwise_output, W, att_weight)
    except Exception:
        try:
            out = _run_on_device(atomwise_output, W, att_weight)
        except Exception:
            if os.environ.get("KERNEL_NO_SUBPROC"):
                raise
            out = _run_in_subprocess(atomwise_output, n_atoms_i, W, att_weight)
    return out.astype(np.float32)


# revision 16
# speedup vs baseline: 1.3202x; 1.3202x over previous
"""AttentionPool kernel for Trainium2, 8 NeuronCores (SPMD data-parallel).

Reference computation (per graph g with atoms A_g, uniform |A_g| = 32):
    h = X @ W.T                              [131072, 512]
    s = leakyrelu(sum(att * h, -1), 0.2)     [131072]
    w = segment_softmax(s)                   per graph
    out[g] = sum_{a in A_g} w[a] * h[a]      [4096, 512]

Algebraic refactor (pool-first; avoids the 69-GFLOP h matmul AND any
transpose of X):
    v  = W.T @ att  (host input prep, tiny)
    s  = lrelu(X @ v)        fused per-tile dot product (DVE/GpSimd stt)
    e  = exp(s)              no max-subtraction needed (|s| <~ 8)
    P[g] = sum_{a in A_g} e[a] X[a]   PE matmul per 128-atom tile with a
                             [128,32] masked-exp stationary em32 built by
                             ACT: exp(maskbias + s); 8-tile batches write
                             a 32-aligned PSUM partition window
    d[g] = per-tile matmul em32.T @ ones (ap_size=1, ~free on PE)
    out = (P / d) @ W.T      per-core projection (PE transposes + matmul)

All heavy data in bf16 (X converted host-side -> 17 MB DMA per core,
matmuls at 1 cyc/row); s/den/PSUM accumulate in fp32. Rel err ~1e-3
vs the 2e-2 gate.

Sharding: 8 cores x 16384 atoms (= 512 graphs, graph-aligned). W/att
replicated. Output slices concatenated on host. Non-uniform segment sizes
fall back to an exact numpy path (never triggered by the fixed harness
inputs, which are uniform 32 atoms/graph).
"""

import numpy as np

N_ATOMS = 131072
FEAT = 512
N_GRAPHS = 4096
NEG_SLOPE = 0.2
N_CORES = 8

P = 128                      # partitions / atoms per tile
NA_CORE = N_ATOMS // N_CORES         # 16384 atoms per core
NT = NA_CORE // P                    # 128 tiles per core
NG_CORE = N_GRAPHS // N_CORES        # 512 graphs per core
GPT = P // 32                        # 4 graphs per tile (uniform 32 atoms/graph)
TPG = P // GPT                       # 32 tiles per 128-graph group
NGRP = NT // TPG                     # 4 groups of 128 graphs per core
FCH = FEAT // P                      # 4 feature chunks
DMA_GRP = 8                          # X tiles per input DMA (1 MiB in bf16)
W32 = 8 * GPT                        # stationary width = graphs per 8-tile batch
GP_KS = (2, 6)                       # tiles per 8-batch whose product runs on GpSimd

_CACHED = {}


def _build_program():
    import concourse.bacc as bacc
    import concourse.mybir as mybir
    import concourse.tile as tile
    from concourse.masks import make_identity
    from contextlib import ExitStack

    F32 = mybir.dt.float32
    BF16 = mybir.dt.bfloat16
    FP16 = mybir.dt.float16
    MULT = mybir.AluOpType.mult
    ADD = mybir.AluOpType.add
    MAX = mybir.AluOpType.max
    EXP = mybir.ActivationFunctionType.Exp

    nc = bacc.Bacc("TRN2", target_bir_lowering=False, debug=False,
                   num_devices=N_CORES)

    COPY = mybir.ActivationFunctionType.Copy

    x = nc.dram_tensor("x", [NA_CORE, FEAT], BF16, kind="ExternalInput").ap()
    wt = nc.dram_tensor("wt", [FEAT, FEAT], BF16, kind="ExternalInput").ap()
    vrep = nc.dram_tensor("vrep", [P, FEAT], BF16, kind="ExternalInput").ap()
    mb256 = nc.dram_tensor("mb256", [P, 8, W32], FP16,
                           kind="ExternalInput").ap()
    out = nc.dram_tensor("out", [NG_CORE, FEAT], F32, kind="ExternalOutput").ap()

    x_r8 = x.rearrange("(n o p) f -> n p o f", o=DMA_GRP, p=P)
    x_r4 = x.rearrange("(n o p) f -> n p o f", o=4, p=P)

    with tile.TileContext(nc) as tc, ExitStack() as ctx:
        singles = ctx.enter_context(tc.tile_pool(name="singles", bufs=1))
        xpool = ctx.enter_context(tc.tile_pool(name="xpool", bufs=6))
        x4pool = ctx.enter_context(tc.tile_pool(name="x4pool", bufs=2))
        spool = ctx.enter_context(tc.tile_pool(name="spool", bufs=6))
        prodv = ctx.enter_context(tc.tile_pool(name="prodv", bufs=4))
        prodg = ctx.enter_context(tc.tile_pool(name="prodg", bufs=3))
        junkp = ctx.enter_context(tc.tile_pool(name="junkp", bufs=2))
        empool = ctx.enter_context(tc.tile_pool(name="empool", bufs=8))
        smallp = ctx.enter_context(tc.tile_pool(name="smallp", bufs=4))
        pooledp = ctx.enter_context(tc.tile_pool(name="pooledp", bufs=2))
        ptp = ctx.enter_context(tc.tile_pool(name="ptp", bufs=4))
        outp = ctx.enter_context(tc.tile_pool(name="outp", bufs=2))
        ps_pool = ctx.enter_context(tc.tile_pool(name="ps_pool", bufs=2, space="PSUM"))
        ps_den = ctx.enter_context(tc.tile_pool(name="ps_den", bufs=2, space="PSUM"))
        ps_misc = ctx.enter_context(tc.tile_pool(name="ps_misc", bufs=2, space="PSUM"))
        ps_out = ctx.enter_context(tc.tile_pool(name="ps_out", bufs=2, space="PSUM"))

        # ---- first X chunk + constants, then the streaming 8-tile chunks ----
        xc0 = x4pool.tile([P, 4, FEAT], BF16, tag="x4i", name="xc0")
        nc.sync.dma_start(out=xc0, in_=x_r4[0])
        v_rep = singles.tile([P, FEAT], BF16)
        nc.sync.dma_start(out=v_rep, in_=vrep)
        xc1 = x4pool.tile([P, 4, FEAT], BF16, tag="x4i", name="xc1")
        nc.sync.dma_start(out=xc1, in_=x_r4[1])
        mb_sb = singles.tile([P, 8, W32], FP16)
        nc.sync.dma_start(out=mb_sb, in_=mb256)
        wt_sb = singles.tile([P, FCH, FEAT], BF16)
        nc.sync.dma_start(out=wt_sb, in_=wt.rearrange("(c p) f -> p c f", p=P))
        ident = singles.tile([P, P], BF16)
        make_identity(nc, ident)
        ones_col = singles.tile([P, 1], BF16)
        nc.vector.memset(ones_col, 1.0)

        # warm up the PE (HAM clock gate) while the score pipeline fills
        warm_ps = ps_out.tile([P, FEAT], F32, tag="ops", name="warm_ps")
        for wi in range(6):
            nc.tensor.matmul(warm_ps, lhsT=ident, rhs=v_rep,
                             start=(wi == 0), stop=(wi == 5))

        state = {"chunk": None}

        def xref(t):
            """SBUF AP for tile t; issues the owning chunk DMA at boundaries."""
            if t < 4:
                return xc0[:, t, :]
            if t < 8:
                return xc1[:, t - 4, :]
            n, o = divmod(t, DMA_GRP)
            if o == 0:
                state["chunk"] = xpool.tile([P, DMA_GRP, FEAT], BF16, tag="x8",
                                            name=f"x8_{n}")
                nc.sync.dma_start(out=state["chunk"], in_=x_r8[n])
            return state["chunk"][:, o, :]

        def emit_batch(g, bu, pool_ps, den_ps):
            s_b = spool.tile([P, 8], F32, tag="s_b")
            xts = [xref(g * TPG + bu * 8 + k) for k in range(8)]
            # GpSimd products issued first so they're done before ACT reduces
            prods = {}
            for k in GP_KS:
                pg = prodg.tile([P, FEAT], FP16, tag="pg")
                nc.gpsimd.tensor_tensor(out=pg, in0=xts[k], in1=v_rep, op=MULT)
                prods[k] = pg
            for k in range(8):
                if k in GP_KS:
                    junka = prodv.tile([P, FEAT], FP16, tag="junka")
                    nc.scalar.activation(out=junka, in_=prods[k], func=COPY,
                                         accum_out=s_b[:, k:k + 1])
                else:
                    junk = junkp.tile([P, FEAT], BF16, tag="junk")
                    nc.vector.scalar_tensor_tensor(
                        out=junk, in0=xts[k], scalar=1.0, in1=v_rep,
                        op0=MULT, op1=MULT, accum_out=s_b[:, k:k + 1])
            s_lr = spool.tile([P, 8], F32, tag="s_lr")
            nc.vector.scalar_tensor_tensor(
                out=s_lr, in0=s_b, scalar=NEG_SLOPE, in1=s_b,
                op0=MULT, op1=MAX)
            # all 8 masked-exp stationaries in one DVE add + one ACT exp:
            # em256[p, 32k + c] = exp(mb256[p,k,c] + s_lr[p,k]);
            # mb256 is 0 at c == 4k + p//32 and -60000 elsewhere
            madd = empool.tile([P, 8, W32], FP16, tag="madd")
            s_bc = s_lr.rearrange("p (k o) -> p k o", o=1).broadcast_to(
                [P, 8, W32])
            nc.vector.tensor_tensor(out=madd, in0=mb_sb, in1=s_bc, op=ADD)
            em256 = empool.tile([P, 8, W32], BF16, tag="em256")
            nc.scalar.activation(out=em256, in_=madd, func=EXP, scale=1.0)
            win = pool_ps[bu * W32:(bu + 1) * W32, :]
            dwin = den_ps[bu * W32:(bu + 1) * W32, :]
            for k in range(8):
                nc.tensor.matmul(win, lhsT=em256[:, k, :], rhs=xts[k],
                                 start=(k == 0), stop=(k == 7),
                                 tile_position=(0, bu * W32))
                nc.tensor.matmul(dwin, lhsT=em256[:, k, :], rhs=ones_col,
                                 start=(k == 0), stop=(k == 7),
                                 tile_position=(0, bu * W32))

        def emit_tail(g, pool_ps, den_ps):
            # ---- normalize + projection: out[g] = (pool/den) @ W.T ----
            denr = smallp.tile([P, 1], F32, tag="denr")
            nc.vector.reciprocal(denr, den_ps)
            pooled = pooledp.tile([P, FEAT], BF16, tag="pooled")
            out_ps = ps_out.tile([P, FEAT], F32, tag="ops")
            for c in range(FCH):
                nc.scalar.mul(pooled[:, c * P:(c + 1) * P],
                              pool_ps[:, c * P:(c + 1) * P], denr)
                tr_ps = ps_misc.tile([P, P], BF16, tag="tr", name="tr_ps")
                nc.tensor.transpose(tr_ps,
                                    pooled[:, c * P:(c + 1) * P],
                                    ident)
                pt = ptp.tile([P, P], BF16, tag="pt")
                nc.scalar.copy(out=pt, in_=tr_ps)
                nc.tensor.matmul(out_ps, lhsT=pt,
                                 rhs=wt_sb[:, c, :],
                                 start=(c == 0), stop=(c == FCH - 1))
            out_sb = outp.tile([P, FEAT], F32, tag="out_sb")
            nc.scalar.copy(out=out_sb, in_=out_ps)
            nc.sync.dma_start(out=out[g * P:(g + 1) * P, :], in_=out_sb)

        # ---- main loop: 4 groups x 4 batches x 8 tiles, tails pipelined ----
        prev = None
        for g in range(NGRP):
            pool_ps = ps_pool.tile([P, FEAT], F32)
            den_ps = ps_den.tile([P, 1], F32)
            for bu in range(TPG // 8):
                emit_batch(g, bu, pool_ps, den_ps)
                if bu == 0 and prev is not None:
                    emit_tail(*prev)
            prev = (g, pool_ps, den_ps)
        emit_tail(*prev)
    nc.compile()
    return nc


def _host_inputs(atomwise_output, W, att_weight):
    """Per-core input maps (host prep: bf16 conversion + tiny mask tables)."""
    import ml_dtypes
    BF = ml_dtypes.bfloat16
    X = np.asarray(atomwise_output, dtype=np.float32)
    Xb = X.astype(BF)
    Wc = np.ascontiguousarray(np.asarray(W, dtype=np.float32))
    Wt = np.ascontiguousarray(Wc.T)
    att = np.asarray(att_weight, dtype=np.float32)
    v = Wt @ att                                               # v = W.T @ att
    Wtb = Wt.astype(BF)
    vrep = np.ascontiguousarray(np.broadcast_to(v, (P, FEAT))).astype(BF)
    # mask-bias table: mb256[p, k, c] = 0 iff c == 4k + p//32 else -60000
    # (fp16-safe large negative; exp(-60000 + s) == 0)
    pp = np.arange(P)[:, None, None]
    kk = np.arange(8)[None, :, None]
    cc = np.arange(W32)[None, None, :]
    mb = np.where(cc == GPT * kk + pp // 32, 0.0, -60000.0).astype(np.float16)
    mb = np.ascontiguousarray(mb)
    in_maps = []
    for c in range(N_CORES):
        xc = Xb[c * NA_CORE:(c + 1) * NA_CORE]
        in_maps.append({"x": xc, "wt": Wtb, "vrep": vrep, "mb256": mb})
    return in_maps


def _kernel_numpy_fallback(atomwise_output, n_atoms_i, W, att_weight):
    """Exact reference semantics in numpy (used only for non-uniform segments)."""
    X = np.asarray(atomwise_output, dtype=np.float32)
    n_at = np.asarray(n_atoms_i).astype(np.int64)
    W = np.asarray(W, dtype=np.float32)
    att = np.asarray(att_weight, dtype=np.float32)
    h = X @ W.T
    s = (att * h).sum(-1)
    s = np.where(s >= 0, s, NEG_SLOPE * s)
    seg = np.repeat(np.arange(len(n_at)), n_at)[:len(s)]
    ngr = len(n_at)
    smax = np.full(ngr, -np.inf, dtype=np.float32)
    np.maximum.at(smax, seg, s)
    e = np.exp(s - smax[seg])
    den = np.zeros(ngr, dtype=np.float32)
    np.add.at(den, seg, e)
    wgt = e / den[seg]
    outp = np.zeros((ngr, h.shape[1]), dtype=np.float32)
    np.add.at(outp, seg, wgt[:, None] * h)
    return outp


def _run_on_device(atomwise_output, W, att_weight):
    from concourse.bass_utils import run_bass_kernel_spmd

    if "nc" not in _CACHED:
        _CACHED["nc"] = _build_program()
    nc = _CACHED["nc"]
    in_maps = _host_inputs(atomwise_output, W, att_weight)
    res = run_bass_kernel_spmd(nc, in_maps, list(range(N_CORES)))
    return np.concatenate([res.results[c]["out"] for c in range(N_CORES)], axis=0)


def _run_in_subprocess(atomwise_output, n_atoms_i, W, att_weight):
    """Last-resort retry in a fresh process: a transient
    NRT_EXEC_UNIT_UNRECOVERABLE wedges the current NRT client session, but a
    new process (fresh axon boot) recovers. Arrays go via a temp dir."""
    import os, subprocess, sys, tempfile
    kdir = os.path.dirname(os.path.abspath(__file__))
    with tempfile.TemporaryDirectory() as td:
        np.save(os.path.join(td, "x.npy"), np.asarray(atomwise_output))
        np.save(os.path.join(td, "n.npy"), np.asarray(n_atoms_i))
        np.save(os.path.join(td, "w.npy"), np.asarray(W))
        np.save(os.path.join(td, "a.npy"), np.asarray(att_weight))
        driver = (
            "import sys, os, numpy as np\n"
            f"sys.path.insert(0, {kdir!r})\n"
            "import kernel\n"
            f"td = {td!r}\n"
            "out = kernel.kernel(np.load(td+'/x.npy'), np.load(td+'/n.npy'),\n"
            "                    np.load(td+'/w.npy'), np.load(td+'/a.npy'))\n"
            "np.save(td+'/out.npy', out)\n"
        )
        env = dict(os.environ, KERNEL_NO_SUBPROC="1")
        subprocess.run([sys.executable, "-c", driver], env=env, check=True,
                       timeout=1800)
        return np.load(os.path.join(td, "out.npy"))


def kernel(atomwise_output, n_atoms_i, W, att_weight):
    import os
    n_at = np.asarray(n_atoms_i)
    uniform = (
        atomwise_output.shape == (N_ATOMS, FEAT)
        and n_at.shape == (N_GRAPHS,)
        and np.all(n_at == N_ATOMS // N_GRAPHS)
    )
    if not uniform:
        return _kernel_numpy_fallback(atomwise_output, n_atoms_i, W, att_weight)

    try:
        out = _run_on_device(atomwise_output, W, att_weight)
    except Exception:
        try:
            out = _run_on_device(atomwise_output, W, att_weight)
        except Exception:
            if os.environ.get("KERNEL_NO_SUBPROC"):
                raise
            out = _run_in_subprocess(atomwise_output, n_atoms_i, W, att_weight)
    return out.astype(np.float32)
